# revision 27
# baseline (speedup 1.0000x reference)
"""Trainium2 Bass kernel for PhraseAttention.

Computes, per batch item b:
    scores  = ctx[b] @ qry[b] + log(ctx_mask[b])          # (S,)
    alphas  = softmax(scores)                             # (S,)
    parent  = prevatts[b, parent_ptr[b], :]               # (S,)
    alphas  = min(parent, alphas)                         # (S,)
    summary = alphas @ ctx[b]                             # (D,)

Sharding: data-parallel over the batch dim, 4 batch items per core on 8
NeuronCores.  Each core streams its 32MB slice of ctx into SBUF exactly once
(single pass over HBM): the score matvec runs on the Vector engine
(fused multiply+reduce via scalar_tensor_tensor) while tiles arrive, and the
weighted sum reuses the resident tiles on the Tensor engine after softmax.

Layout on chip: per batch item, the S=2048 positions live as a [128, 16]
tile (partition p, column i <-> s = i*128 + p).  All DRAM transfers are
contiguous (>=512B per descriptor); layout changes happen on chip via PE
transposes against an identity matrix.  Softmax uses a per-partition max +
exp (ACT, fused accumulate); the two cross-partition reductions (global
max, global sum) use GpSimd partition_all_reduce which leaves the result
broadcast across partitions.
"""

import numpy as np

B, S, D, T = 32, 2048, 1024, 8
NCORES = 8
BPC = B // NCORES  # batch items per core
P = 128
NS = S // P        # 16 s-chunks per batch item
CPT = 4            # s-chunks per ctx DMA/tile (2MB per DMA)
NT = NS // CPT     # ctx tiles per batch item

_CACHE = {}


def _build(stage=4):
    import os
    from contextlib import ExitStack

    stage = int(os.environ.get("KSTAGE", stage))
    reps = int(os.environ.get("BENCH_REPS", "1"))
    cpt = int(os.environ.get("KCPT", CPT))
    nt = NS // cpt

    import concourse.bass as bass
    import concourse.bass_isa as bass_isa
    import concourse.tile as tile
    from concourse import bacc, mybir
    from concourse.masks import make_identity

    f32 = mybir.dt.float32
    i32 = mybir.dt.int32
    Alu = mybir.AluOpType
    Act = mybir.ActivationFunctionType
    X = mybir.AxisListType.X

    nc = bacc.Bacc("TRN2", target_bir_lowering=False, debug=False)

    qry = nc.dram_tensor("qry", [BPC, D], f32, kind="ExternalInput").ap()
    ctx_in = nc.dram_tensor("ctx", [BPC, S, D], f32, kind="ExternalInput").ap()
    mask_in = nc.dram_tensor("ctx_mask", [BPC, S], f32, kind="ExternalInput").ap()
    pv_in = nc.dram_tensor("prevatts", [BPC, T, S], f32, kind="ExternalInput").ap()
    ptr_in = nc.dram_tensor("parent_ptr", [BPC], i32, kind="ExternalInput").ap()
    alphas_o = nc.dram_tensor("alphas", [BPC, S], f32, kind="ExternalOutput").ap()
    summary_o = nc.dram_tensor("summary", [BPC, D], f32, kind="ExternalOutput").ap()
    scores_o = nc.dram_tensor("scores", [BPC, S], f32, kind="ExternalOutput").ap()

    with tile.TileContext(nc) as tc, ExitStack() as ex:
        singles = ex.enter_context(tc.tile_pool(name="singles", bufs=1))
        ctxp = ex.enter_context(tc.tile_pool(name="ctxp", bufs=(9 if cpt == 4 else 4)))
        qp = ex.enter_context(tc.tile_pool(name="qp", bufs=3))
        scrp = ex.enter_context(tc.tile_pool(name="scrp", bufs=3))
        smp = ex.enter_context(tc.tile_pool(name="smp", bufs=3))
        psp = ex.enter_context(tc.tile_pool(name="psp", bufs=2, space="PSUM"))

        # one-time constants
        identity = singles.tile([P, P], f32)
        make_identity(nc, identity)
        iot = singles.tile([1, T], i32)
        nc.gpsimd.iota(iot, pattern=[[1, T]], base=0, channel_multiplier=0)
        iot_f = singles.tile([1, T], f32)
        nc.vector.tensor_copy(iot_f, iot)
        ones_col = singles.tile([P, 1], f32)
        nc.vector.memset(ones_col, 1.0)
        ones_row = singles.tile([1, P], f32)
        nc.vector.memset(ones_row, 1.0)

        if reps > 1:
            loop_cm = tc.For_i(0, reps, 1)
            loop_cm.__enter__()

        # ---- per-iteration contiguous input DMAs (ACT HWDGE ring) ----
        # pa_all[p, b, q] = prevatts[b, p//16, (p%16)*128 + q]  (p = t*16+i)
        pa_all = smp.tile([P, BPC, P], f32, tag="pa", bufs=2)
        nc.gpsimd.dma_start(
            out=pa_all,
            in_=bass.AP(tensor=pv_in.tensor, offset=pv_in.offset,
                        ap=[[P, P], [T * S, BPC], [1, P]]),
        )
        # mask_all[j, b, q] = ctx_mask[b, j*128 + q]
        mask_all = smp.tile([NS, BPC, P], f32, tag="mask", bufs=2)
        nc.gpsimd.dma_start(
            out=mask_all,
            in_=bass.AP(tensor=mask_in.tensor, offset=mask_in.offset,
                        ap=[[P, NS], [S, BPC], [1, P]]),
        )
        ptr_all = smp.tile([1, BPC], i32, tag="ptri")
        nc.gpsimd.dma_start(out=ptr_all, in_=ptr_in.rearrange("(o b) -> o b", o=1))
        ptr_f = smp.tile([1, BPC], f32, tag="ptrf")
        nc.vector.tensor_copy(ptr_f, ptr_all)

        # qry rows -> broadcast to 128 partitions via PE (off the batch critical path)
        qbc_all = qp.tile([P, BPC, D], f32, tag="qbc", bufs=1)
        for bb in range(BPC):
            qrow = smp.tile([1, D], f32, tag="qrow", bufs=1)
            nc.scalar.dma_start(out=qrow, in_=qry[bb : bb + 1, :])
            for h in range(D // 512):
                qps = psp.tile([P, 512], f32, tag="qps")
                nc.tensor.matmul(
                    qps, lhsT=ones_row, rhs=qrow[:, h * 512 : (h + 1) * 512],
                    start=True, stop=True,
                )
                nc.scalar.copy(qbc_all[:, bb, h * 512 : (h + 1) * 512], qps)

        state = {}

        def emit_load(b):
            qbc = qbc_all[:, b, :]
            # ---- big ctx DMAs (SP HWDGE ring), 2MB each ----
            ctx_r = ctx_in[b].rearrange("(t c p) d -> t p c d", p=P, c=cpt)
            ctiles = []
            for t in range(nt):
                ct = ctxp.tile([P, cpt, D], f32, tag="ctx")
                nc.sync.dma_start(out=ct, in_=ctx_r[t])
                ctiles.append(ct)

            # ---- scores: fused multiply + row-reduce on DVE ----
            scores_sb = smp.tile([P, NS], f32, tag="scores")
            state[b] = (ctiles, scores_sb)
            if stage < 1:
                # DMA-roofline probe: touch each ctx tile with a tiny reduce
                for t in range(nt):
                    for c in range(cpt):
                        i = t * cpt + c
                        nc.vector.tensor_reduce(
                            out=scores_sb[:, i : i + 1],
                            in_=ctiles[t][:, c, 0:16], axis=X, op=Alu.max)
                sT = psp.tile([NS, P], f32, tag="outT", bufs=2)
                nc.tensor.transpose(sT, scores_sb, identity)
                sT_sb = smp.tile([NS, P], f32, tag="scoresT_sb")
                nc.scalar.copy(sT_sb, sT)
                nc.scalar.dma_start(
                    out=scores_o[b].rearrange("(i q) -> i q", i=NS), in_=sT_sb)
                nc.scalar.dma_start(
                    out=alphas_o[b].rearrange("(i q) -> i q", i=NS), in_=sT_sb)
                nc.scalar.dma_start(out=summary_o[b : b + 1, 0:NS], in_=scores_sb[0:1, 0:NS])
                return

            for t in range(nt):
                for c in range(cpt):
                    i = t * cpt + c
                    scr = scrp.tile([P, D], f32, tag="scr")
                    nc.vector.scalar_tensor_tensor(
                        out=scr,
                        in0=ctiles[t][:, c, :],
                        scalar=1.0,
                        in1=qbc,
                        op0=Alu.mult,
                        op1=Alu.mult,
                        accum_out=scores_sb[:, i : i + 1],
                    )

            if stage == 1:
                scoresT1 = psp.tile([NS, P], f32, tag="outT", bufs=2)
                nc.tensor.transpose(scoresT1, scores_sb, identity)
                sT1_sb = smp.tile([NS, P], f32, tag="scoresT_sb")
                nc.scalar.copy(sT1_sb, scoresT1)
                nc.scalar.dma_start(
                    out=scores_o[b].rearrange("(i q) -> i q", i=NS), in_=sT1_sb)
                nc.scalar.dma_start(
                    out=alphas_o[b].rearrange("(i q) -> i q", i=NS), in_=sT1_sb)
                nc.scalar.dma_start(out=summary_o[b : b + 1, 0:NS], in_=scores_sb[0:1, 0:NS])
                return

        def emit_post(b):
            ctiles, scores_sb = state.pop(b)
            # scores += log(mask)   (mask arrives [NS, P]; PE-transpose it)
            maskT = psp.tile([P, NS], f32, tag="tp")
            nc.tensor.transpose(maskT, mask_all[:, b, :], identity[0:NS, 0:NS])
            logm = smp.tile([P, NS], f32, tag="logm")
            nc.scalar.activation(out=logm, in_=maskT, func=Act.Ln)
            nc.vector.tensor_add(scores_sb, scores_sb, logm)

            # scores out: PE-transpose to [NS, P] so the DMA is contiguous
            scoresT = psp.tile([NS, P], f32, tag="outT", bufs=2)
            nc.tensor.transpose(scoresT, scores_sb, identity)
            scoresT_sb = smp.tile([NS, P], f32, tag="scoresT_sb")
            nc.scalar.copy(scoresT_sb, scoresT)
            nc.gpsimd.dma_start(
                out=scores_o[b].rearrange("(i q) -> i q", i=NS), in_=scoresT_sb
            )

            # ---- softmax (per-partition max, then global rescale) ----
            m = smp.tile([P, 1], f32, tag="m")
            nc.vector.tensor_reduce(out=m, in_=scores_sb, axis=X, op=Alu.max)
            nm = smp.tile([P, 1], f32, tag="nm")
            nc.vector.tensor_reduce(out=nm, in_=scores_sb, axis=X, op=Alu.max, negate=True)
            e = smp.tile([P, NS], f32, tag="e")
            srow = smp.tile([P, 1], f32, tag="srow")
            nc.scalar.activation(
                out=e, in_=scores_sb, func=Act.Exp, bias=nm, scale=1.0, accum_out=srow
            )
            mbc = smp.tile([P, 1], f32, tag="mbc")
            nc.gpsimd.partition_all_reduce(
                mbc, m, channels=P, reduce_op=bass_isa.ReduceOp.max
            )
            nmbc = smp.tile([P, 1], f32, tag="nmbc")
            nc.vector.tensor_scalar_mul(nmbc, mbc, -1.0)
            w = smp.tile([P, 1], f32, tag="w")
            nc.scalar.activation(out=w, in_=m, func=Act.Exp, bias=nmbc, scale=1.0)
            tz = smp.tile([P, 1], f32, tag="tz")
            nc.vector.tensor_mul(tz, srow, w)
            z = smp.tile([P, 1], f32, tag="z")
            nc.gpsimd.partition_all_reduce(
                z, tz, channels=P, reduce_op=bass_isa.ReduceOp.add
            )
            rz = smp.tile([P, 1], f32, tag="rz")
            nc.vector.reciprocal(rz, z)
            fac = smp.tile([P, 1], f32, tag="fac")
            nc.vector.tensor_mul(fac, w, rz)

            # ---- parent gather via one-hot over T ----
            # pa_all[:, b, :] is [t*16+i, q]; PE-transpose -> paT[q, t*16+i]
            paT = psp.tile([P, P], f32, tag="tp")
            nc.tensor.transpose(paT, pa_all[:, b, :], identity)
            oh = smp.tile([1, T], f32, tag="oh")
            nc.vector.tensor_scalar(
                oh, iot_f, ptr_f[0:1, b : b + 1], None, op0=Alu.is_equal
            )
            ohb = smp.tile([P, T], f32, tag="ohb")
            nc.gpsimd.partition_broadcast(ohb, oh)
            pav = bass.AP(tensor=paT.tensor, offset=paT.offset,
                          ap=[paT.ap[0], [1, NS], [NS, T]])
            ohv = bass.AP(tensor=ohb.tensor, offset=ohb.offset,
                          ap=[ohb.ap[0], [0, NS], [1, T]])
            ptmp = smp.tile([P, NS, T], f32, tag="ptmp")
            nc.vector.tensor_tensor(ptmp, pav, ohv, op=Alu.mult)
            pa_sel = smp.tile([P, NS], f32, tag="pasel")
            nc.vector.tensor_reduce(out=pa_sel, in_=ptmp, axis=X, op=Alu.add)

            # alphas = min(parent_alphas, softmax_alphas)
            alphas_sb = smp.tile([P, NS], f32, tag="alphas")
            nc.vector.scalar_tensor_tensor(
                out=alphas_sb, in0=e, scalar=fac[:, 0:1], in1=pa_sel,
                op0=Alu.mult, op1=Alu.min,
            )

            alphasT = psp.tile([NS, P], f32, tag="outT", bufs=2)
            nc.tensor.transpose(alphasT, alphas_sb, identity)
            alphasT_sb = smp.tile([NS, P], f32, tag="alphasT_sb")
            nc.scalar.copy(alphasT_sb, alphasT)
            nc.gpsimd.dma_start(
                out=alphas_o[b].rearrange("(i q) -> i q", i=NS), in_=alphasT_sb
            )

            # ---- summary = alphas @ ctx: chunks 0..KPE-1 on PE, rest on DVE ----
            KPE = NS if stage >= 5 else (4 if b == BPC - 1 else 6)
            acc = None
            if KPE < NS:
                acc = scrp.tile([P, D], f32, tag="acc", bufs=2)
            for i in range(KPE, NS):
                t, c = i // cpt, i % cpt
                if i == KPE:
                    nc.vector.tensor_scalar_mul(
                        acc, ctiles[t][:, c, :], alphas_sb[:, i : i + 1]
                    )
                else:
                    nc.vector.scalar_tensor_tensor(
                        out=acc, in0=ctiles[t][:, c, :],
                        scalar=alphas_sb[:, i : i + 1], in1=acc,
                        op0=Alu.mult, op1=Alu.add,
                    )
            summary_sb = smp.tile([1, D], f32, tag="summ", bufs=2)
            for h in range(D // 512):
                ps = psp.tile([1, 512], f32, tag="ps", bufs=2)
                for i in range(KPE):
                    t, c = i // cpt, i % cpt
                    nc.tensor.matmul(
                        ps,
                        lhsT=alphas_sb[:, i : i + 1],
                        rhs=ctiles[t][:, c, h * 512 : (h + 1) * 512],
                        start=(i == 0),
                        stop=(KPE == NS and i == NS - 1),
                    )
                if KPE < NS:
                    nc.tensor.matmul(
                        ps,
                        lhsT=ones_col,
                        rhs=acc[:, h * 512 : (h + 1) * 512],
                        start=False,
                        stop=True,
                    )
                nc.scalar.copy(summary_sb[:, h * 512 : (h + 1) * 512], ps)
            nc.gpsimd.dma_start(out=summary_o[b : b + 1, :], in_=summary_sb)

        for b in range(BPC):
            emit_load(b)
            if stage >= 2 and b >= 1:
                emit_post(b - 1)
        if stage >= 2:
            emit_post(BPC - 1)

        if reps > 1:
            loop_cm.__exit__(None, None, None)

    nc.compile()
    return nc


def _get_compiled():
    if "nc" not in _CACHE:
        _CACHE["nc"] = _build()
    return _CACHE["nc"]


def kernel(qry, ctx, ctx_mask, prevatts, parent_ptr):
    from concourse.bass_utils import run_bass_kernel_spmd

    nc = _get_compiled()
    qry = np.ascontiguousarray(np.asarray(qry), dtype=np.float32)
    ctx = np.ascontiguousarray(np.asarray(ctx), dtype=np.float32)
    ctx_mask = np.ascontiguousarray(np.asarray(ctx_mask), dtype=np.float32)
    prevatts = np.ascontiguousarray(np.asarray(prevatts), dtype=np.float32)
    parent_ptr = np.ascontiguousarray(np.asarray(parent_ptr), dtype=np.int32)

    in_maps = []
    for c in range(NCORES):
        sl = slice(c * BPC, (c + 1) * BPC)
        in_maps.append(
            {
                "qry": qry[sl],
                "ctx": ctx[sl],
                "ctx_mask": ctx_mask[sl],
                "prevatts": prevatts[sl],
                "parent_ptr": parent_ptr[sl],
            }
        )
    res = run_bass_kernel_spmd(nc, in_maps, core_ids=list(range(NCORES)))
    alphas = np.concatenate([res.results[c]["alphas"] for c in range(NCORES)], axis=0)
    summary = np.concatenate([res.results[c]["summary"] for c in range(NCORES)], axis=0)
    scores = np.concatenate([res.results[c]["scores"] for c in range(NCORES)], axis=0)
    return alphas, summary, scores
